# revision 29
# baseline (speedup 1.0000x reference)
"""Criss-Cross Attention TRN2 Bass kernel (v2).

Problem: x[16,512,96,96]; q,k = 1x1 conv to 64ch; v = 1x1 conv to 512ch;
column+row criss-cross softmax attention (column set excludes the center
pixel); out = gamma * agg + x.

Sharding: data-parallel over batch, 2 batches per core on 8 cores.

v3 changes vs baseline:
- The softmax normalization (and gamma, now applied on device) folds
  into the attention transpose: pat = expH^T @ diag(gamma/D) computed
  as a regular PE matmul with a diagonal rhs, eliminating the 384
  per-column/row DVE scale ops. The [96,2,96] diag tiles are built by
  cheap Pool-engine affine_selects from a broadcast AP.
  (fp8 projections were tried and abandoned: per-element fp8 rounding
  gives ~3.7% error on q/k/v that does NOT average down - q,k errors
  blow up through exp into ~60% attention-weight error.)
- Phase-P x loads batched to 768-pixel DMAs (half the DMA count,
  1.5KB contiguous runs).
- Output combine uses the fused scalar_tensor_tensor for both row
  pairs (drops an ACT copy per quad).

Per-core dataflow (per batch):
  P: stream x (f16) in 768-pixel blocks; project q,k (weight-stationary
     f16 matmuls, fp32 psum) and v; the row-energy (eW) group for the
     rows each 384-px half completes is computed inline.
  E: column energies eH per 4-column group: PE matmul -> ACT exp (bf16,
     unnormalized) -> gpsimd affine_select zeroes the u==h center ->
     DVE row-sum. D = SH + SW^T (PE transpose); rDg = gamma/D;
     diag3[p,w,j] = rDg[p,w]*(p==j) built on Pool.
  C: per column pair: PE-transpose v slices (psum f16) -> evac
     (DVE/ACT alternating); attT = (expH slice)^T @ diag3 slice (PE,
     fused normalize+transpose) -> evac; agg matmuls -> psum [c,h];
     evac to o_col f16.
  R: diag3 rebuilt from rDg^T; per 4-row quad (2 pairs): same
     transposes/agg; o_col rows accumulate into the same psum group via
     identity matmuls; combine+x-residual via fused DVE ops; one f16
     out-DMA per quad (768B runs).

Output is f16; the host upcasts to f32.
"""

import numpy as np

import concourse.bass as bass
import concourse.mybir as mybir
import concourse.tile as tile
from concourse import bacc
from concourse.alu_op_type import AluOpType
from concourse.masks import make_identity

F16 = mybir.dt.float16
F32 = mybir.dt.float32
BF16 = mybir.dt.bfloat16
AF = mybir.ActivationFunctionType

B, C, H, W = 16, 512, 96, 96
CQK = 64
HW = H * W
NCORES = 8
BLOC = B // NCORES  # batches per core
KCH = 4  # C / 128 channel chunks
PXB = 384  # pixel block for projections
NPXB = HW // PXB
LDB = 384  # pixel block per phase-P DMA load
WG = 4  # columns/rows per energy group


def build_nc(bloc=BLOC, reps=1, dbg=False):
    nc = bacc.Bacc()

    x16 = nc.declare_dram_parameter("x16", [bloc, C, H, W], F16, isOutput=False)
    wqkT = nc.declare_dram_parameter("wqkT", [C, 2 * CQK], F16, isOutput=False)
    wvT = nc.declare_dram_parameter("wvT", [C, C], F16, isOutput=False)
    bqk = nc.declare_dram_parameter("bqk", [2 * CQK], F32, isOutput=False)
    bv = nc.declare_dram_parameter("bv", [C], F32, isOutput=False)
    gamma = nc.declare_dram_parameter("gamma", [1], F32, isOutput=False)
    out = nc.declare_dram_parameter("out", [bloc, C, H, W], F16, isOutput=True)
    if dbg:
        dbg_qk = nc.declare_dram_parameter("dbg_qk", [CQK, 2, HW], F16, isOutput=True)
        dbg_v = nc.declare_dram_parameter("dbg_v", [128, KCH, HW], F16, isOutput=True)
        dbg_rdg = nc.declare_dram_parameter("dbg_rdg", [96, W], F32, isOutput=True)
        dbg_eh = nc.declare_dram_parameter("dbg_eh", [96, W, H], BF16, isOutput=True)
        dbg_ocol = nc.declare_dram_parameter(
            "dbg_ocol", [128, KCH, H, W], F16, isOutput=True
        )

    x16ap = x16[:]
    outap = out[:]

    with tile.TileContext(nc) as tc:
        with (
            tc.tile_pool(name="cn", bufs=1) as cn,
            tc.tile_pool(name="big", bufs=1) as big,
            tc.tile_pool(name="att", bufs=1) as att,
            tc.tile_pool(name="sm", bufs=1) as sm,
            tc.tile_pool(name="st", bufs=2) as st,
            tc.tile_pool(name="wkv", bufs=2) as wkv,
            tc.tile_pool(name="wk", bufs=2) as wk,
            tc.tile_pool(name="pp", bufs=2, space="PSUM") as pp,
        ):
            # ---- constants ----
            id128 = cn.tile([128, 128], F16, tag="id128")
            make_identity(nc, id128)
            id96f = cn.tile([96, 96], F32, tag="id96f")
            make_identity(nc, id96f)

            wqkT_sb = cn.tile([128, KCH, 2 * CQK], F16, tag="wqkT")
            nc.sync.dma_start(
                out=wqkT_sb,
                in_=bass.AP(
                    tensor=wqkT[:].tensor,
                    offset=wqkT[:].offset,
                    ap=[[2 * CQK, 128], [128 * 2 * CQK, KCH], [1, 2 * CQK]],
                ),
            )
            wvT_sb = cn.tile([128, KCH, C], F16, tag="wvT")
            nc.sync.dma_start(
                out=wvT_sb,
                in_=bass.AP(
                    tensor=wvT[:].tensor,
                    offset=wvT[:].offset,
                    ap=[[C, 128], [128 * C, KCH], [1, C]],
                ),
            )
            bq_sb = cn.tile([CQK, 1], F32, tag="bq")
            nc.sync.dma_start(
                out=bq_sb,
                in_=bass.AP(
                    tensor=bqk[:].tensor, offset=bqk[:].offset, ap=[[1, CQK], [1, 1]]
                ),
            )
            bk_sb = cn.tile([CQK, 1], F32, tag="bk")
            nc.sync.dma_start(
                out=bk_sb,
                in_=bass.AP(
                    tensor=bqk[:].tensor,
                    offset=bqk[:].offset + CQK,
                    ap=[[1, CQK], [1, 1]],
                ),
            )
            bv_sb = cn.tile([128, KCH], F32, tag="bv")
            nc.sync.dma_start(
                out=bv_sb,
                in_=bass.AP(
                    tensor=bv[:].tensor, offset=bv[:].offset, ap=[[1, 128], [128, KCH]]
                ),
            )
            gam_sb = cn.tile([96, 1], F32, tag="gam")
            nc.sync.dma_start(
                out=gam_sb,
                in_=bass.AP(
                    tensor=gamma[:].tensor, offset=gamma[:].offset, ap=[[0, 96], [1, 1]]
                ),
            )

            for b in [b for _ in range(reps) for b in range(bloc)]:
                # ---------- Phase P: projections (+ interleaved eW groups) ----------
                qk_sb = big.tile([CQK, 2, HW], F16, tag="big")  # [:,0]=q, [:,1]=k
                v_sb = big.tile([128, KCH, H, W], F16, tag="v")
                expH = att.tile([96, W, H], BF16, tag="eh")  # [h, w, u]
                expW = att.tile([96, H, W], BF16, tag="ew")  # [w, h, v]
                SH = sm.tile([96, W], F32, tag="SH")  # [h, w]
                SW = sm.tile([96, H], F32, tag="SW")  # [w, h]
                rDg = sm.tile([96, W], F32, tag="rDg")  # gamma/D
                rDg_bf = sm.tile([96, W], BF16, tag="rDgbf")
                q3 = qk_sb[:, 0, :].rearrange("c (h w) -> c h w", w=W)
                k3 = qk_sb[:, 1, :].rearrange("c (h w) -> c h w", w=W)
                for j2 in range(HW // LDB):
                    xs2 = st.tile([128, KCH, LDB], F16, tag="xs")
                    nc.sync.dma_start(
                        out=xs2,
                        in_=bass.AP(
                            tensor=x16ap.tensor,
                            offset=x16ap.offset + b * C * HW + j2 * LDB,
                            ap=[[HW, 128], [128 * HW, KCH], [1, LDB]],
                        ),
                    )
                    for jh in range(LDB // PXB):
                        j = j2 * (LDB // PXB) + jh
                        xs = xs2[:, :, jh * PXB : (jh + 1) * PXB]
                        pq = pp.tile([CQK, PXB], F32, tag="pT")
                        for k in range(KCH):
                            nc.tensor.matmul(
                                pq,
                                wqkT_sb[:, k, 0:CQK],
                                xs[:, k, :],
                                start=(k == 0),
                                stop=(k == KCH - 1),
                            )
                        nc.scalar.activation(
                            out=qk_sb[:, 0, j * PXB : (j + 1) * PXB],
                            in_=pq,
                            func=AF.Identity,
                            bias=bq_sb,
                        )
                        pk = pp.tile([CQK, PXB], F32, tag="pT")
                        for k in range(KCH):
                            nc.tensor.matmul(
                                pk,
                                wqkT_sb[:, k, CQK : 2 * CQK],
                                xs[:, k, :],
                                start=(k == 0),
                                stop=(k == KCH - 1),
                            )
                        nc.vector.tensor_scalar_add(
                            qk_sb[:, 1, j * PXB : (j + 1) * PXB], pk, bk_sb
                        )
                        for m in range(KCH):
                            pv = pp.tile([128, PXB], F32, tag="pCE")
                            for k in range(KCH):
                                nc.tensor.matmul(
                                    pv,
                                    wvT_sb[:, k, 128 * m : 128 * (m + 1)],
                                    xs[:, k, :],
                                    start=(k == 0),
                                    stop=(k == KCH - 1),
                                )
                            vdst = v_sb.rearrange("p m h w -> p m (h w)")[
                                :, m, j * PXB : (j + 1) * PXB
                            ]
                            if m % 2 == 0:
                                nc.vector.tensor_scalar_add(
                                    vdst, pv, bv_sb[:, m : m + 1]
                                )
                            else:
                                nc.scalar.activation(
                                    out=vdst,
                                    in_=pv,
                                    func=AF.Identity,
                                    bias=bv_sb[:, m : m + 1],
                                )
                        # eW energy group for the 4 rows this half completes
                        pe = pp.tile([96, WG, 96], F32, tag="pCE")
                        for i in range(WG):
                            h = j * WG + i
                            nc.tensor.matmul(
                                pe[:, i, :],
                                q3[:, h, :],
                                k3[:, h, :],
                                start=True,
                                stop=True,
                            )
                        dst = expW[:, j * WG : (j + 1) * WG, :]
                        nc.scalar.activation(out=dst, in_=pe, func=AF.Exp)
                        nc.vector.tensor_reduce(
                            out=SW[:, j * WG : (j + 1) * WG],
                            in_=dst,
                            op=AluOpType.add,
                            axis=mybir.AxisListType.X,
                        )

                # ---------- Phase E: column energies ----------
                for eg in range(W // WG):
                    pe = pp.tile([96, WG, 96], F32, tag="pCE")
                    for i in range(WG):
                        w = eg * WG + i
                        nc.tensor.matmul(
                            pe[:, i, :], q3[:, :, w], k3[:, :, w], start=True, stop=True
                        )
                    dst = expH[:, eg * WG : (eg + 1) * WG, :]
                    nc.scalar.activation(out=dst, in_=pe, func=AF.Exp)
                    nc.gpsimd.affine_select(
                        out=dst,
                        in_=dst,
                        compare_op=AluOpType.not_equal,
                        fill=0.0,
                        base=0,
                        pattern=[[0, WG], [-1, 96]],
                        channel_multiplier=1,
                    )
                    nc.vector.tensor_reduce(
                        out=SH[:, eg * WG : (eg + 1) * WG],
                        in_=dst,
                        op=AluOpType.add,
                        axis=mybir.AxisListType.X,
                    )
                # D = SH + SW^T ; rDg = gamma / D
                pt = pp.tile([96, 96], F32, tag="pCE")
                nc.tensor.transpose(pt, SW, id96f)
                nc.vector.tensor_tensor(out=SH, in0=SH, in1=pt, op=AluOpType.add)
                nc.vector.reciprocal(rDg, SH)
                nc.vector.tensor_scalar_mul(rDg, rDg, gam_sb)
                pt2 = pp.tile([96, 96], F32, tag="pCE")
                nc.tensor.transpose(pt2, rDg, id96f)
                rDgT = sm.tile([96, H], F32, tag="rDgT")
                nc.vector.tensor_copy(rDgT, pt2)
                nc.vector.tensor_copy(rDg_bf, rDg)
                if dbg and b == 0:
                    nc.sync.dma_start(out=dbg_qk[:], in_=qk_sb)
                    nc.sync.dma_start(
                        out=dbg_v[:], in_=v_sb.rearrange("p m h w -> p m (h w)")
                    )
                    nc.sync.dma_start(out=dbg_rdg[:], in_=rDg)
                    nc.sync.dma_start(out=dbg_eh[:], in_=expH)

                # ---------- Phase C: column pass (pairs of columns) ----------
                o_col = big.tile([128, KCH, H, W], F16, tag="big")
                for g in range(W // 2):
                    w0 = 2 * g
                    pvt = pp.tile([96, 2, KCH, 128], F16, tag="pT")
                    for wi in range(2):
                        for k in range(KCH):
                            nc.tensor.transpose(
                                pvt[:, wi, k, :], v_sb[:, k, :, w0 + wi], id128
                            )
                    vt1 = wkv.tile([96, 2, KCH, 128], F16, tag="vt")
                    if g % 2 == 0:
                        nc.vector.tensor_copy(vt1, pvt)
                    else:
                        nc.scalar.copy(vt1, pvt)
                    # attT = (expH slice)^T @ diag(rDg slice): normalize+transpose
                    # diag2[p, wi, j] = rDg[p, w0+wi] * (p == j), built on Pool
                    diag2 = wk.tile([96, 2, 96], BF16, tag="diag")
                    nc.gpsimd.affine_select(
                        out=diag2,
                        in_=rDg_bf[:, w0 : w0 + 2].unsqueeze(2).broadcast_to((96, 2, 96)),
                        compare_op=AluOpType.is_equal,
                        fill=0.0,
                        base=0,
                        pattern=[[0, 2], [-1, 96]],
                        channel_multiplier=1,
                    )
                    pat = pp.tile([96, 2, 96], F32, tag="pCE")
                    for wi in range(2):
                        nc.tensor.matmul(
                            pat[:, wi, :],
                            expH[:, w0 + wi, :],
                            diag2[:, wi, :],
                            start=True,
                            stop=True,
                        )
                    attT = wk.tile([96, 2, 96], F16, tag="attT")
                    if g % 2 == 0:
                        nc.scalar.copy(attT, pat)
                    else:
                        nc.vector.tensor_copy(attT, pat)
                    pagg = pp.tile([128, KCH, 2, 128], F32, tag="pAGG")
                    for m in range(KCH):
                        for wi in range(2):
                            nc.tensor.matmul(
                                pagg[:, m, wi, 0:96],
                                vt1[:, wi, m, :],
                                attT[:, wi, :],
                                start=True,
                                stop=True,
                            )
                    srcA = pagg[:, :, :, 0:96].rearrange("p m wi h -> p m h wi")
                    dstA = o_col[:, :, :, w0 : w0 + 2]
                    if g % 2 == 0:
                        nc.scalar.copy(dstA, srcA)
                    else:
                        nc.vector.tensor_copy(dstA, srcA)

                if dbg and b == 0:
                    nc.sync.dma_start(
                        out=dbg_ocol[:],
                        in_=o_col.rearrange("p m h w -> p m (h w)").rearrange(
                            "p m (h w) -> p m h w", w=W
                        ),
                    )
                # ---------- Phase R: row pass (quads = 2 pairs of rows) ----------
                rDgT_bf = sm.tile([96, H], BF16, tag="rDgTbf")
                nc.vector.tensor_copy(rDgT_bf, rDgT)
                for q in range(H // 4):
                    h0q = 4 * q
                    orow = st.tile([128, KCH, 4, 96], F16, tag="orow")
                    xrow = st.tile([128, KCH, 384], F16, tag="xs")
                    nc.sync.dma_start(
                        out=xrow,
                        in_=bass.AP(
                            tensor=x16ap.tensor,
                            offset=x16ap.offset + b * C * HW + h0q * W,
                            ap=[[HW, 128], [128 * HW, KCH], [1, 384]],
                        ),
                    )
                    for p in range(2):
                        h0 = h0q + 2 * p
                        pvt = pp.tile([96, 2, KCH, 128], F16, tag="pT")
                        for hi in range(2):
                            for k in range(KCH):
                                nc.tensor.transpose(
                                    pvt[:, hi, k, :], v_sb[:, k, h0 + hi, :], id128
                                )
                        vt2 = wkv.tile([96, 2, KCH, 128], F16, tag="vt")
                        nc.scalar.copy(vt2, pvt)
                        diag2r = wk.tile([96, 2, 96], BF16, tag="diag")
                        nc.gpsimd.affine_select(
                            out=diag2r,
                            in_=rDgT_bf[:, h0 : h0 + 2]
                            .unsqueeze(2)
                            .broadcast_to((96, 2, 96)),
                            compare_op=AluOpType.is_equal,
                            fill=0.0,
                            base=0,
                            pattern=[[0, 2], [-1, 96]],
                            channel_multiplier=1,
                        )
                        pat2 = pp.tile([96, 2, 96], F32, tag="pCE")
                        for hi in range(2):
                            nc.tensor.matmul(
                                pat2[:, hi, :],
                                expW[:, h0 + hi, :],
                                diag2r[:, hi, :],
                                start=True,
                                stop=True,
                            )
                        attT2 = wk.tile([96, 2, 96], F16, tag="attT")
                        nc.scalar.copy(attT2, pat2)
                        pagg2 = pp.tile([128, KCH, 2, 128], F32, tag="pAGG")
                        for m in range(KCH):
                            for hi in range(2):
                                nc.tensor.matmul(
                                    pagg2[:, m, hi, 0:96],
                                    vt2[:, hi, m, :],
                                    attT2[:, hi, :],
                                    start=True,
                                    stop=True,
                                )
                        odst = orow[:, :, 2 * p : 2 * p + 2, :]
                        xsl = xrow.rearrange("p m (hi w) -> p m hi w", hi=4)[
                            :, :, 2 * p : 2 * p + 2, :
                        ]
                        # orow = (pagg2 + o_col_rows) + xrow: two DVE passes
                        # (the o_col accumulation moved off PE - identity
                        # matmuls cost ~81ns each on HW at N=96)
                        nc.vector.scalar_tensor_tensor(
                            out=odst,
                            in0=pagg2[:, :, :, 0:96],
                            scalar=1.0,
                            in1=o_col[:, :, h0 : h0 + 2, :],
                            op0=AluOpType.mult,
                            op1=AluOpType.add,
                        )
                        nc.vector.tensor_tensor(
                            out=odst, in0=odst, in1=xsl, op=AluOpType.add
                        )
                    nc.sync.dma_start(
                        out=bass.AP(
                            tensor=outap.tensor,
                            offset=outap.offset + b * C * HW + h0q * W,
                            ap=[[HW, 128], [128 * HW, KCH], [1, 384]],
                        ),
                        in_=orow.rearrange("p m hi w -> p m (hi w)"),
                    )
    nc.finalize()
    return nc


_NC_CACHE = {}


def _get_nc():
    if "nc" not in _NC_CACHE:
        _NC_CACHE["nc"] = build_nc()
    return _NC_CACHE["nc"]


def make_in_maps(x, Wq, bq, Wk, bk, Wv, bv, gamma):
    x = np.asarray(x, dtype=np.float32)
    gamma = np.asarray(gamma, dtype=np.float32)
    wqkT = np.ascontiguousarray(
        np.concatenate([np.asarray(Wq), np.asarray(Wk)], axis=0).T
    ).astype(np.float16)
    wvT = np.ascontiguousarray(np.asarray(Wv).T).astype(np.float16)
    bqk = np.concatenate([np.asarray(bq), np.asarray(bk)]).astype(np.float32)
    bv = np.asarray(bv, dtype=np.float32)
    x16 = x.astype(np.float16)
    in_maps = []
    for c in range(NCORES):
        sl = slice(c * BLOC, (c + 1) * BLOC)
        in_maps.append(
            {
                "x16": x16[sl],
                "wqkT": wqkT,
                "wvT": wvT,
                "bqk": bqk,
                "bv": bv,
                "gamma": gamma,
            }
        )
    return in_maps


def kernel(x, Wq, bq, Wk, bk, Wv, bv, gamma):
    from concourse.bass_utils import run_bass_kernel_spmd

    nc = _get_nc()
    in_maps = make_in_maps(x, Wq, bq, Wk, bk, Wv, bv, gamma)
    res = run_bass_kernel_spmd(nc, in_maps, core_ids=list(range(NCORES)))
    return np.concatenate([r["out"] for r in res.results], axis=0).astype(np.float32)


# revision 31
# speedup vs baseline: 2.6355x; 2.6355x over previous
"""Criss-Cross Attention TRN2 Bass kernel (v2).

Problem: x[16,512,96,96]; q,k = 1x1 conv to 64ch; v = 1x1 conv to 512ch;
column+row criss-cross softmax attention (column set excludes the center
pixel); out = gamma * agg + x.

Sharding: data-parallel over batch, 2 batches per core on 8 cores.

v3 changes vs baseline:
- The softmax normalization (and gamma, now applied on device) folds
  into the attention transpose: pat = expH^T @ diag(gamma/D) computed
  as a regular PE matmul with a diagonal rhs, eliminating the 384
  per-column/row DVE scale ops. The [96,2,96] diag tiles are built by
  cheap Pool-engine affine_selects from a broadcast AP.
  (fp8 projections were tried and abandoned: per-element fp8 rounding
  gives ~3.7% error on q/k/v that does NOT average down - q,k errors
  blow up through exp into ~60% attention-weight error.)
- Phase-P x loads batched to 768-pixel DMAs (half the DMA count,
  1.5KB contiguous runs).
- Output combine uses the fused scalar_tensor_tensor for both row
  pairs (drops an ACT copy per quad).

Per-core dataflow (per batch):
  P: stream x (f16) in 768-pixel blocks; project q,k (weight-stationary
     f16 matmuls, fp32 psum) and v; the row-energy (eW) group for the
     rows each 384-px half completes is computed inline.
  E: column energies eH per 4-column group: PE matmul -> ACT exp (bf16,
     unnormalized) -> gpsimd affine_select zeroes the u==h center ->
     DVE row-sum. D = SH + SW^T (PE transpose); rDg = gamma/D;
     diag3[p,w,j] = rDg[p,w]*(p==j) built on Pool.
  C: per column pair: PE-transpose v slices (psum f16) -> evac
     (DVE/ACT alternating); attT = (expH slice)^T @ diag3 slice (PE,
     fused normalize+transpose) -> evac; agg matmuls -> psum [c,h];
     evac to o_col f16.
  R: diag3 rebuilt from rDg^T; per 4-row quad (2 pairs): same
     transposes/agg; o_col rows accumulate into the same psum group via
     identity matmuls; combine+x-residual via fused DVE ops; one f16
     out-DMA per quad (768B runs).

Output is f16; the host upcasts to f32.
"""

import numpy as np

import concourse.bass as bass
import concourse.mybir as mybir
import concourse.tile as tile
from concourse import bacc
from concourse.alu_op_type import AluOpType
from concourse.masks import make_identity

F16 = mybir.dt.float16
F32 = mybir.dt.float32
BF16 = mybir.dt.bfloat16
AF = mybir.ActivationFunctionType

B, C, H, W = 16, 512, 96, 96
CQK = 64
HW = H * W
NCORES = 8
BLOC = B // NCORES  # batches per core
KCH = 4  # C / 128 channel chunks
PXB = 384  # pixel block for projections
NPXB = HW // PXB
LDB = 384  # pixel block per phase-P DMA load
WG = 4  # columns/rows per energy group


def build_nc(bloc=BLOC, reps=1, dbg=False):
    nc = bacc.Bacc()

    x16 = nc.declare_dram_parameter("x16", [bloc, C, H, W], F16, isOutput=False)
    wqkT = nc.declare_dram_parameter("wqkT", [C, 2 * CQK], F16, isOutput=False)
    wvT = nc.declare_dram_parameter("wvT", [C, C], F16, isOutput=False)
    bqk = nc.declare_dram_parameter("bqk", [2 * CQK], F32, isOutput=False)
    bv = nc.declare_dram_parameter("bv", [C], F32, isOutput=False)
    gamma = nc.declare_dram_parameter("gamma", [1], F32, isOutput=False)
    out = nc.declare_dram_parameter("out", [bloc, C, H, W], F16, isOutput=True)
    if dbg:
        dbg_qk = nc.declare_dram_parameter("dbg_qk", [CQK, 2, HW], F16, isOutput=True)
        dbg_v = nc.declare_dram_parameter("dbg_v", [128, KCH, HW], F16, isOutput=True)
        dbg_rdg = nc.declare_dram_parameter("dbg_rdg", [96, W], F32, isOutput=True)
        dbg_eh = nc.declare_dram_parameter("dbg_eh", [96, W, H], BF16, isOutput=True)
        dbg_ocol = nc.declare_dram_parameter(
            "dbg_ocol", [128, KCH, H, W], F16, isOutput=True
        )

    x16ap = x16[:]
    outap = out[:]

    with tile.TileContext(nc) as tc:
        with (
            tc.tile_pool(name="cn", bufs=1) as cn,
            tc.tile_pool(name="big", bufs=1) as big,
            tc.tile_pool(name="att", bufs=1) as att,
            tc.tile_pool(name="sm", bufs=1) as sm,
            tc.tile_pool(name="st", bufs=2) as st,
            tc.tile_pool(name="wkv", bufs=2) as wkv,
            tc.tile_pool(name="wk", bufs=2) as wk,
            tc.tile_pool(name="pp", bufs=2, space="PSUM") as pp,
        ):
            # ---- constants ----
            id128 = cn.tile([128, 128], F16, tag="id128")
            make_identity(nc, id128)
            id96f = cn.tile([96, 96], F32, tag="id96f")
            make_identity(nc, id96f)

            wqkT_sb = cn.tile([128, KCH, 2 * CQK], F16, tag="wqkT")
            nc.sync.dma_start(
                out=wqkT_sb,
                in_=bass.AP(
                    tensor=wqkT[:].tensor,
                    offset=wqkT[:].offset,
                    ap=[[2 * CQK, 128], [128 * 2 * CQK, KCH], [1, 2 * CQK]],
                ),
            )
            wvT_sb = cn.tile([128, KCH, C], F16, tag="wvT")
            nc.sync.dma_start(
                out=wvT_sb,
                in_=bass.AP(
                    tensor=wvT[:].tensor,
                    offset=wvT[:].offset,
                    ap=[[C, 128], [128 * C, KCH], [1, C]],
                ),
            )
            bq_sb = cn.tile([CQK, 1], F32, tag="bq")
            nc.sync.dma_start(
                out=bq_sb,
                in_=bass.AP(
                    tensor=bqk[:].tensor, offset=bqk[:].offset, ap=[[1, CQK], [1, 1]]
                ),
            )
            bk_sb = cn.tile([CQK, 1], F32, tag="bk")
            nc.sync.dma_start(
                out=bk_sb,
                in_=bass.AP(
                    tensor=bqk[:].tensor,
                    offset=bqk[:].offset + CQK,
                    ap=[[1, CQK], [1, 1]],
                ),
            )
            bv_sb = cn.tile([128, KCH], F32, tag="bv")
            nc.sync.dma_start(
                out=bv_sb,
                in_=bass.AP(
                    tensor=bv[:].tensor, offset=bv[:].offset, ap=[[1, 128], [128, KCH]]
                ),
            )
            gam_sb = cn.tile([96, 1], F32, tag="gam")
            nc.sync.dma_start(
                out=gam_sb,
                in_=bass.AP(
                    tensor=gamma[:].tensor, offset=gamma[:].offset, ap=[[0, 96], [1, 1]]
                ),
            )

            for b in [b for _ in range(reps) for b in range(bloc)]:
                # ---------- Phase P: projections (+ interleaved eW groups) ----------
                qk_sb = big.tile([CQK, 2, HW], F16, tag="big")  # [:,0]=q, [:,1]=k
                v_sb = big.tile([128, KCH, H, W], F16, tag="v")
                expH = att.tile([96, W, H], BF16, tag="eh")  # [h, w, u]
                expW = att.tile([96, H, W], BF16, tag="ew")  # [w, h, v]
                SH = sm.tile([96, W], F32, tag="SH")  # [h, w]
                SW = sm.tile([96, H], F32, tag="SW")  # [w, h]
                rDg = sm.tile([96, W], F32, tag="rDg")  # gamma/D
                rDg_bf = sm.tile([96, W], BF16, tag="rDgbf")
                q3 = qk_sb[:, 0, :].rearrange("c (h w) -> c h w", w=W)
                k3 = qk_sb[:, 1, :].rearrange("c (h w) -> c h w", w=W)
                for j2 in range(HW // LDB):
                    xs2 = st.tile([128, KCH, LDB], F16, tag="xs")
                    nc.sync.dma_start(
                        out=xs2,
                        in_=bass.AP(
                            tensor=x16ap.tensor,
                            offset=x16ap.offset + b * C * HW + j2 * LDB,
                            ap=[[HW, 128], [128 * HW, KCH], [1, LDB]],
                        ),
                    )
                    for jh in range(LDB // PXB):
                        j = j2 * (LDB // PXB) + jh
                        xs = xs2[:, :, jh * PXB : (jh + 1) * PXB]
                        pq = pp.tile([CQK, PXB], F32, tag="pT")
                        for k in range(KCH):
                            nc.tensor.matmul(
                                pq,
                                wqkT_sb[:, k, 0:CQK],
                                xs[:, k, :],
                                start=(k == 0),
                                stop=(k == KCH - 1),
                            )
                        nc.scalar.activation(
                            out=qk_sb[:, 0, j * PXB : (j + 1) * PXB],
                            in_=pq,
                            func=AF.Identity,
                            bias=bq_sb,
                        )
                        pk = pp.tile([CQK, PXB], F32, tag="pT")
                        for k in range(KCH):
                            nc.tensor.matmul(
                                pk,
                                wqkT_sb[:, k, CQK : 2 * CQK],
                                xs[:, k, :],
                                start=(k == 0),
                                stop=(k == KCH - 1),
                            )
                        nc.vector.tensor_scalar_add(
                            qk_sb[:, 1, j * PXB : (j + 1) * PXB], pk, bk_sb
                        )
                        for m in range(KCH):
                            pv = pp.tile([128, PXB], F32, tag="pCE")
                            for k in range(KCH):
                                nc.tensor.matmul(
                                    pv,
                                    wvT_sb[:, k, 128 * m : 128 * (m + 1)],
                                    xs[:, k, :],
                                    start=(k == 0),
                                    stop=(k == KCH - 1),
                                )
                            vdst = v_sb.rearrange("p m h w -> p m (h w)")[
                                :, m, j * PXB : (j + 1) * PXB
                            ]
                            if m % 2 == 0:
                                nc.vector.tensor_scalar_add(
                                    vdst, pv, bv_sb[:, m : m + 1]
                                )
                            else:
                                nc.scalar.activation(
                                    out=vdst,
                                    in_=pv,
                                    func=AF.Identity,
                                    bias=bv_sb[:, m : m + 1],
                                )
                        # eW energy group for the 4 rows this half completes
                        pe = pp.tile([96, WG, 96], F32, tag="pCE")
                        for i in range(WG):
                            h = j * WG + i
                            nc.tensor.matmul(
                                pe[:, i, :],
                                q3[:, h, :],
                                k3[:, h, :],
                                start=True,
                                stop=True,
                            )
                        dst = expW[:, j * WG : (j + 1) * WG, :]
                        nc.scalar.activation(out=dst, in_=pe, func=AF.Exp)
                        nc.vector.tensor_reduce(
                            out=SW[:, j * WG : (j + 1) * WG],
                            in_=dst,
                            op=AluOpType.add,
                            axis=mybir.AxisListType.X,
                        )

                # ---------- Phase E: column energies ----------
                for eg in range(W // WG):
                    pe = pp.tile([96, WG, 96], F32, tag="pCE")
                    for i in range(WG):
                        w = eg * WG + i
                        nc.tensor.matmul(
                            pe[:, i, :], q3[:, :, w], k3[:, :, w], start=True, stop=True
                        )
                    dst = expH[:, eg * WG : (eg + 1) * WG, :]
                    nc.scalar.activation(out=dst, in_=pe, func=AF.Exp)
                    nc.gpsimd.affine_select(
                        out=dst,
                        in_=dst,
                        compare_op=AluOpType.not_equal,
                        fill=0.0,
                        base=0,
                        pattern=[[0, WG], [-1, 96]],
                        channel_multiplier=1,
                    )
                    nc.vector.tensor_reduce(
                        out=SH[:, eg * WG : (eg + 1) * WG],
                        in_=dst,
                        op=AluOpType.add,
                        axis=mybir.AxisListType.X,
                    )
                # D = SH + SW^T ; rDg = gamma / D
                pt = pp.tile([96, 96], F32, tag="pCE")
                nc.tensor.transpose(pt, SW, id96f)
                nc.vector.tensor_tensor(out=SH, in0=SH, in1=pt, op=AluOpType.add)
                nc.vector.reciprocal(rDg, SH)
                nc.vector.tensor_scalar_mul(rDg, rDg, gam_sb)
                pt2 = pp.tile([96, 96], F32, tag="pCE")
                nc.tensor.transpose(pt2, rDg, id96f)
                rDgT = sm.tile([96, H], F32, tag="rDgT")
                nc.vector.tensor_copy(rDgT, pt2)
                nc.vector.tensor_copy(rDg_bf, rDg)
                if dbg and b == 0:
                    nc.sync.dma_start(out=dbg_qk[:], in_=qk_sb)
                    nc.sync.dma_start(
                        out=dbg_v[:], in_=v_sb.rearrange("p m h w -> p m (h w)")
                    )
                    nc.sync.dma_start(out=dbg_rdg[:], in_=rDg)
                    nc.sync.dma_start(out=dbg_eh[:], in_=expH)

                # ---------- Phase C: column pass (4-col attT groups, col pairs) ----------
                o_col = big.tile([128, KCH, H, W], F16, tag="big")
                for g2 in range(W // 4):
                    w00 = 4 * g2
                    # attT = (expH slice)^T @ diag(rDg slice): normalize+transpose
                    # diag4[p, wi, j] = rDg[p, w00+wi] * (p == j), built on Pool
                    diag4 = wk.tile([96, 4, 96], BF16, tag="diag")
                    nc.gpsimd.affine_select(
                        out=diag4,
                        in_=rDg_bf[:, w00 : w00 + 4]
                        .unsqueeze(2)
                        .broadcast_to((96, 4, 96)),
                        compare_op=AluOpType.is_equal,
                        fill=0.0,
                        base=0,
                        pattern=[[0, 4], [-1, 96]],
                        channel_multiplier=1,
                    )
                    pat = pp.tile([96, 4, 96], F32, tag="pCE")
                    for wi in range(4):
                        nc.tensor.matmul(
                            pat[:, wi, :],
                            expH[:, w00 + wi, :],
                            diag4[:, wi, :],
                            start=True,
                            stop=True,
                        )
                    attT = wk.tile([96, 4, 96], F16, tag="attT")
                    if g2 % 2 == 0:
                        nc.scalar.copy(attT, pat)
                    else:
                        nc.vector.tensor_copy(attT, pat)
                    for gg in range(2):
                        g = 2 * g2 + gg
                        w0 = 4 * g2 + 2 * gg
                        pvt = pp.tile([96, 2, KCH, 128], F16, tag="pT")
                        for wi in range(2):
                            for k in range(KCH):
                                nc.tensor.transpose(
                                    pvt[:, wi, k, :], v_sb[:, k, :, w0 + wi], id128
                                )
                        vt1 = wkv.tile([96, 2, KCH, 128], F16, tag="vt")
                        if g % 2 == 0:
                            nc.vector.tensor_copy(vt1, pvt)
                        else:
                            nc.scalar.copy(vt1, pvt)
                        pagg = pp.tile([128, KCH, 2, 128], F32, tag="pAGG")
                        for m in range(KCH):
                            for wi in range(2):
                                nc.tensor.matmul(
                                    pagg[:, m, wi, 0:96],
                                    vt1[:, wi, m, :],
                                    attT[:, 2 * gg + wi, :],
                                    start=True,
                                    stop=True,
                                )
                        srcA = pagg[:, :, :, 0:96].rearrange("p m wi h -> p m h wi")
                        dstA = o_col[:, :, :, w0 : w0 + 2]
                        if g % 2 == 0:
                            nc.scalar.copy(dstA, srcA)
                        else:
                            nc.vector.tensor_copy(dstA, srcA)

                if dbg and b == 0:
                    nc.sync.dma_start(
                        out=dbg_ocol[:],
                        in_=o_col.rearrange("p m h w -> p m (h w)").rearrange(
                            "p m (h w) -> p m h w", w=W
                        ),
                    )
                # ---------- Phase R: row pass (quads = 2 pairs of rows) ----------
                rDgT_bf = sm.tile([96, H], BF16, tag="rDgTbf")
                nc.vector.tensor_copy(rDgT_bf, rDgT)
                for q in range(H // 4):
                    h0q = 4 * q
                    orow = st.tile([128, KCH, 4, 96], F16, tag="orow")
                    xrow = st.tile([128, KCH, 384], F16, tag="xs")
                    nc.sync.dma_start(
                        out=xrow,
                        in_=bass.AP(
                            tensor=x16ap.tensor,
                            offset=x16ap.offset + b * C * HW + h0q * W,
                            ap=[[HW, 128], [128 * HW, KCH], [1, 384]],
                        ),
                    )
                    diag4r = wk.tile([96, 4, 96], BF16, tag="diag")
                    nc.gpsimd.affine_select(
                        out=diag4r,
                        in_=rDgT_bf[:, h0q : h0q + 4]
                        .unsqueeze(2)
                        .broadcast_to((96, 4, 96)),
                        compare_op=AluOpType.is_equal,
                        fill=0.0,
                        base=0,
                        pattern=[[0, 4], [-1, 96]],
                        channel_multiplier=1,
                    )
                    pat2 = pp.tile([96, 4, 96], F32, tag="pCE")
                    for hi in range(4):
                        nc.tensor.matmul(
                            pat2[:, hi, :],
                            expW[:, h0q + hi, :],
                            diag4r[:, hi, :],
                            start=True,
                            stop=True,
                        )
                    attT2 = wk.tile([96, 4, 96], F16, tag="attT")
                    nc.scalar.copy(attT2, pat2)
                    for p in range(2):
                        h0 = h0q + 2 * p
                        pvt = pp.tile([96, 2, KCH, 128], F16, tag="pT")
                        for hi in range(2):
                            for k in range(KCH):
                                nc.tensor.transpose(
                                    pvt[:, hi, k, :], v_sb[:, k, h0 + hi, :], id128
                                )
                        vt2 = wkv.tile([96, 2, KCH, 128], F16, tag="vt")
                        nc.scalar.copy(vt2, pvt)
                        pagg2 = pp.tile([128, KCH, 2, 128], F32, tag="pAGG")
                        for m in range(KCH):
                            for hi in range(2):
                                nc.tensor.matmul(
                                    pagg2[:, m, hi, 0:96],
                                    vt2[:, hi, m, :],
                                    attT2[:, 2 * p + hi, :],
                                    start=True,
                                    stop=True,
                                )
                        odst = orow[:, :, 2 * p : 2 * p + 2, :]
                        xsl = xrow.rearrange("p m (hi w) -> p m hi w", hi=4)[
                            :, :, 2 * p : 2 * p + 2, :
                        ]
                        # orow = (pagg2 + o_col_rows) + xrow: two DVE passes
                        # (the o_col accumulation moved off PE - identity
                        # matmuls cost ~81ns each on HW at N=96)
                        nc.vector.scalar_tensor_tensor(
                            out=odst,
                            in0=pagg2[:, :, :, 0:96],
                            scalar=1.0,
                            in1=o_col[:, :, h0 : h0 + 2, :],
                            op0=AluOpType.mult,
                            op1=AluOpType.add,
                        )
                        nc.vector.tensor_tensor(
                            out=odst, in0=odst, in1=xsl, op=AluOpType.add
                        )
                    nc.sync.dma_start(
                        out=bass.AP(
                            tensor=outap.tensor,
                            offset=outap.offset + b * C * HW + h0q * W,
                            ap=[[HW, 128], [128 * HW, KCH], [1, 384]],
                        ),
                        in_=orow.rearrange("p m hi w -> p m (hi w)"),
                    )
    nc.finalize()
    return nc


_NC_CACHE = {}


def _get_nc():
    if "nc" not in _NC_CACHE:
        _NC_CACHE["nc"] = build_nc()
    return _NC_CACHE["nc"]


def make_in_maps(x, Wq, bq, Wk, bk, Wv, bv, gamma):
    x = np.asarray(x, dtype=np.float32)
    gamma = np.asarray(gamma, dtype=np.float32)
    wqkT = np.ascontiguousarray(
        np.concatenate([np.asarray(Wq), np.asarray(Wk)], axis=0).T
    ).astype(np.float16)
    wvT = np.ascontiguousarray(np.asarray(Wv).T).astype(np.float16)
    bqk = np.concatenate([np.asarray(bq), np.asarray(bk)]).astype(np.float32)
    bv = np.asarray(bv, dtype=np.float32)
    x16 = x.astype(np.float16)
    in_maps = []
    for c in range(NCORES):
        sl = slice(c * BLOC, (c + 1) * BLOC)
        in_maps.append(
            {
                "x16": x16[sl],
                "wqkT": wqkT,
                "wvT": wvT,
                "bqk": bqk,
                "bv": bv,
                "gamma": gamma,
            }
        )
    return in_maps


def kernel(x, Wq, bq, Wk, bk, Wv, bv, gamma):
    from concourse.bass_utils import run_bass_kernel_spmd

    nc = _get_nc()
    in_maps = make_in_maps(x, Wq, bq, Wk, bk, Wv, bv, gamma)
    res = run_bass_kernel_spmd(nc, in_maps, core_ids=list(range(NCORES)))
    return np.concatenate([r["out"] for r in res.results], axis=0).astype(np.float32)


# revision 39
# speedup vs baseline: 3.5718x; 1.3553x over previous
"""Criss-Cross Attention TRN2 Bass kernel (v2).

Problem: x[16,512,96,96]; q,k = 1x1 conv to 64ch; v = 1x1 conv to 512ch;
column+row criss-cross softmax attention (column set excludes the center
pixel); out = gamma * agg + x.

Sharding: data-parallel over batch, 2 batches per core on 8 cores.

Changes vs the session-start baseline:
- The softmax normalization (and gamma, now applied on device) folds
  into the attention transpose: attT = expH_slice^T @ diag(gamma/D)
  computed as a regular PE matmul with a diagonal rhs, eliminating the
  384 per-column/row DVE scale ops. The [96,4,96] diag tiles are built
  by cheap Pool-engine affine_selects from a broadcast AP.
  (fp8 projections were tried and abandoned: per-element fp8 rounding
  gives ~3.7% error on q/k/v that does NOT average down - q,k errors
  blow up through exp into ~60% attention-weight error.)
- The o_col accumulation moved off PE (was: 768 identity matmuls/core
  at ~81ns each on HW) into the final DVE combine.
- attT/diag work batched per 4 columns/rows: fewer, larger psum-evac
  ops on ACT/DVE.
- PXB=512 projection blocks (not row-aligned; the inline eW energy
  groups fire from a rows-completed counter): 432 vs 576 projection
  matmuls per batch and 25% fewer phase-P DMAs. Small matmuls cost
  ~81ns on HW regardless of N<=128 (LDWEIGHTS+MM pipeline), so fewer,
  wider matmuls win even though the cost-model sim is indifferent.

Per-core dataflow (per batch):
  P: stream x (f16) in 512-pixel blocks; project q,k (weight-stationary
     f16 matmuls, fp32 psum) and v; eW energy groups fire inline as
     their 4 rows complete.
  E: column energies eH per 4-column group: PE matmul -> ACT exp (bf16,
     unnormalized) -> gpsimd affine_select zeroes the u==h center ->
     DVE row-sum. D = SH + SW^T (PE transpose); rDg = gamma/D;
     diag3[p,w,j] = rDg[p,w]*(p==j) built on Pool.
  C: per column pair: PE-transpose v slices (psum f16) -> evac
     (DVE/ACT alternating); attT = (expH slice)^T @ diag3 slice (PE,
     fused normalize+transpose) -> evac; agg matmuls -> psum [c,h];
     evac to o_col f16.
  R: diag3 rebuilt from rDg^T; per 4-row quad (2 pairs): same
     transposes/agg; o_col rows accumulate into the same psum group via
     identity matmuls; combine+x-residual via fused DVE ops; one f16
     out-DMA per quad (768B runs).

Output is f16; the host upcasts to f32.
"""

import numpy as np

import concourse.bass as bass
import concourse.mybir as mybir
import concourse.tile as tile
from concourse import bacc
from concourse.alu_op_type import AluOpType
from concourse.masks import make_identity

F16 = mybir.dt.float16
F32 = mybir.dt.float32
BF16 = mybir.dt.bfloat16
AF = mybir.ActivationFunctionType

B, C, H, W = 16, 512, 96, 96
CQK = 64
HW = H * W
NCORES = 8
BLOC = B // NCORES  # batches per core
KCH = 4  # C / 128 channel chunks
PXB = 512  # pixel block for projections (not row-aligned; eW decoupled)
NPXB = HW // PXB
WG = 4  # columns/rows per energy group


def build_nc(bloc=BLOC, reps=1, dbg=False):
    nc = bacc.Bacc()

    x16 = nc.declare_dram_parameter("x16", [bloc, C, H, W], F16, isOutput=False)
    wqkT = nc.declare_dram_parameter("wqkT", [C, 2 * CQK], F16, isOutput=False)
    wvT = nc.declare_dram_parameter("wvT", [C, C], F16, isOutput=False)
    bqk = nc.declare_dram_parameter("bqk", [2 * CQK], F32, isOutput=False)
    bv = nc.declare_dram_parameter("bv", [C], F32, isOutput=False)
    gamma = nc.declare_dram_parameter("gamma", [1], F32, isOutput=False)
    out = nc.declare_dram_parameter("out", [bloc, C, H, W], F16, isOutput=True)
    if dbg:
        dbg_qk = nc.declare_dram_parameter("dbg_qk", [CQK, 2, HW], F16, isOutput=True)
        dbg_v = nc.declare_dram_parameter("dbg_v", [128, KCH, HW], F16, isOutput=True)
        dbg_rdg = nc.declare_dram_parameter("dbg_rdg", [96, W], F32, isOutput=True)
        dbg_eh = nc.declare_dram_parameter("dbg_eh", [96, W, H], BF16, isOutput=True)
        dbg_ocol = nc.declare_dram_parameter(
            "dbg_ocol", [128, KCH, H, W], F16, isOutput=True
        )

    x16ap = x16[:]
    outap = out[:]

    with tile.TileContext(nc) as tc:
        with (
            tc.tile_pool(name="cn", bufs=1) as cn,
            tc.tile_pool(name="big", bufs=1) as big,
            tc.tile_pool(name="att", bufs=1) as att,
            tc.tile_pool(name="sm", bufs=1) as sm,
            tc.tile_pool(name="st", bufs=2) as st,
            tc.tile_pool(name="wkv", bufs=2) as wkv,
            tc.tile_pool(name="wk", bufs=2) as wk,
            tc.tile_pool(name="pp", bufs=2, space="PSUM") as pp,
        ):
            # ---- constants ----
            id128 = cn.tile([128, 128], F16, tag="id128")
            make_identity(nc, id128)
            id96f = cn.tile([96, 96], F32, tag="id96f")
            make_identity(nc, id96f)

            wqkT_sb = cn.tile([128, KCH, 2 * CQK], F16, tag="wqkT")
            nc.sync.dma_start(
                out=wqkT_sb,
                in_=bass.AP(
                    tensor=wqkT[:].tensor,
                    offset=wqkT[:].offset,
                    ap=[[2 * CQK, 128], [128 * 2 * CQK, KCH], [1, 2 * CQK]],
                ),
            )
            wvT_sb = cn.tile([128, KCH, C], F16, tag="wvT")
            nc.sync.dma_start(
                out=wvT_sb,
                in_=bass.AP(
                    tensor=wvT[:].tensor,
                    offset=wvT[:].offset,
                    ap=[[C, 128], [128 * C, KCH], [1, C]],
                ),
            )
            bqk_sb = cn.tile([CQK, 2], F32, tag="bq")
            nc.sync.dma_start(
                out=bqk_sb,
                in_=bass.AP(
                    tensor=bqk[:].tensor, offset=bqk[:].offset, ap=[[1, CQK], [CQK, 2]]
                ),
            )
            bq_sb = bqk_sb[:, 0:1]
            bk_sb = bqk_sb[:, 1:2]
            bv_sb = cn.tile([128, KCH], F32, tag="bv")
            nc.sync.dma_start(
                out=bv_sb,
                in_=bass.AP(
                    tensor=bv[:].tensor, offset=bv[:].offset, ap=[[1, 128], [128, KCH]]
                ),
            )
            gam_sb = cn.tile([96, 1], F32, tag="gam")
            nc.sync.dma_start(
                out=gam_sb,
                in_=bass.AP(
                    tensor=gamma[:].tensor, offset=gamma[:].offset, ap=[[0, 96], [1, 1]]
                ),
            )

            for b in [b for _ in range(reps) for b in range(bloc)]:
                # ---------- Phase P: projections (+ interleaved eW groups) ----------
                qk_sb = big.tile([CQK, 2, HW], F16, tag="big")  # [:,0]=q, [:,1]=k
                v_sb = big.tile([128, KCH, H, W], F16, tag="v")
                expH = att.tile([96, W, H], BF16, tag="eh")  # [h, w, u]
                expW = att.tile([96, H, W], BF16, tag="ew")  # [w, h, v]
                SH = sm.tile([96, W], F32, tag="SH")  # [h, w]; becomes gamma/D
                SW = sm.tile([96, H], F32, tag="SW")  # [w, h]
                rDg_bf = sm.tile([96, W], BF16, tag="rDgbf")
                q3 = qk_sb[:, 0, :].rearrange("c (h w) -> c h w", w=W)
                k3 = qk_sb[:, 1, :].rearrange("c (h w) -> c h w", w=W)
                ew_done = 0
                for j in range(NPXB):
                    xs = st.tile([128, KCH, PXB], F16, tag="xs")
                    nc.sync.dma_start(
                        out=xs,
                        in_=bass.AP(
                            tensor=x16ap.tensor,
                            offset=x16ap.offset + b * C * HW + j * PXB,
                            ap=[[HW, 128], [128 * HW, KCH], [1, PXB]],
                        ),
                    )
                    pq = pp.tile([CQK, PXB], F32, tag="pT")
                    for k in range(KCH):
                        nc.tensor.matmul(
                            pq,
                            wqkT_sb[:, k, 0:CQK],
                            xs[:, k, :],
                            start=(k == 0),
                            stop=(k == KCH - 1),
                        )
                    nc.scalar.activation(
                        out=qk_sb[:, 0, j * PXB : (j + 1) * PXB],
                        in_=pq,
                        func=AF.Identity,
                        bias=bq_sb,
                    )
                    pk = pp.tile([CQK, PXB], F32, tag="pT")
                    for k in range(KCH):
                        nc.tensor.matmul(
                            pk,
                            wqkT_sb[:, k, CQK : 2 * CQK],
                            xs[:, k, :],
                            start=(k == 0),
                            stop=(k == KCH - 1),
                        )
                    nc.vector.tensor_scalar_add(
                        qk_sb[:, 1, j * PXB : (j + 1) * PXB], pk, bk_sb
                    )
                    for m in range(KCH):
                        pv = pp.tile([128, PXB], F32, tag="pCE")
                        for k in range(KCH):
                            nc.tensor.matmul(
                                pv,
                                wvT_sb[:, k, 128 * m : 128 * (m + 1)],
                                xs[:, k, :],
                                start=(k == 0),
                                stop=(k == KCH - 1),
                            )
                        vdst = v_sb.rearrange("p m h w -> p m (h w)")[
                            :, m, j * PXB : (j + 1) * PXB
                        ]
                        if m % 2 == 0:
                            nc.vector.tensor_scalar_add(vdst, pv, bv_sb[:, m : m + 1])
                        else:
                            nc.scalar.activation(
                                out=vdst,
                                in_=pv,
                                func=AF.Identity,
                                bias=bv_sb[:, m : m + 1],
                            )
                    # eW energy groups for rows completed by this block
                    rows_done = ((j + 1) * PXB) // W
                    while ew_done + WG <= rows_done:
                        h0e = ew_done
                        pe = pp.tile([96, WG, 96], F32, tag="pCE")
                        for i in range(WG):
                            h = h0e + i
                            nc.tensor.matmul(
                                pe[:, i, :],
                                q3[:, h, :],
                                k3[:, h, :],
                                start=True,
                                stop=True,
                            )
                        dst = expW[:, h0e : h0e + WG, :]
                        nc.scalar.activation(out=dst, in_=pe, func=AF.Exp)
                        nc.vector.tensor_reduce(
                            out=SW[:, h0e : h0e + WG],
                            in_=dst,
                            op=AluOpType.add,
                            axis=mybir.AxisListType.X,
                        )
                        ew_done += WG

                # ---------- Phase E: column energies ----------
                for eg in range(W // WG):
                    pe = pp.tile([96, WG, 96], F32, tag="pCE")
                    for i in range(WG):
                        w = eg * WG + i
                        nc.tensor.matmul(
                            pe[:, i, :], q3[:, :, w], k3[:, :, w], start=True, stop=True
                        )
                    dst = expH[:, eg * WG : (eg + 1) * WG, :]
                    nc.scalar.activation(out=dst, in_=pe, func=AF.Exp)
                    nc.gpsimd.affine_select(
                        out=dst,
                        in_=dst,
                        compare_op=AluOpType.not_equal,
                        fill=0.0,
                        base=0,
                        pattern=[[0, WG], [-1, 96]],
                        channel_multiplier=1,
                    )
                    nc.vector.tensor_reduce(
                        out=SH[:, eg * WG : (eg + 1) * WG],
                        in_=dst,
                        op=AluOpType.add,
                        axis=mybir.AxisListType.X,
                    )
                # D = SH + SW^T ; rDg = gamma / D (in place in SH)
                pt = pp.tile([96, 96], F32, tag="pCE")
                nc.tensor.transpose(pt, SW, id96f)
                nc.vector.tensor_tensor(out=SH, in0=SH, in1=pt, op=AluOpType.add)
                nc.vector.reciprocal(SH, SH)
                nc.vector.tensor_scalar_mul(SH, SH, gam_sb)
                pt2 = pp.tile([96, 96], F32, tag="pCE")
                nc.tensor.transpose(pt2, SH, id96f)
                rDgT_bf = sm.tile([96, H], BF16, tag="rDgTbf")
                nc.vector.tensor_copy(rDgT_bf, pt2)
                nc.vector.tensor_copy(rDg_bf, SH)
                if dbg and b == 0:
                    nc.sync.dma_start(out=dbg_qk[:], in_=qk_sb)
                    nc.sync.dma_start(
                        out=dbg_v[:], in_=v_sb.rearrange("p m h w -> p m (h w)")
                    )
                    nc.sync.dma_start(out=dbg_rdg[:], in_=SH)
                    nc.sync.dma_start(out=dbg_eh[:], in_=expH)

                # ---------- Phase C: column pass (4-col attT groups, col pairs) ----------
                o_col = big.tile([128, KCH, H, W], F16, tag="big")
                for g2 in range(W // 4):
                    w00 = 4 * g2
                    # attT = (expH slice)^T @ diag(rDg slice): normalize+transpose
                    # diag4[p, wi, j] = rDg[p, w00+wi] * (p == j), built on Pool
                    diag4 = wk.tile([96, 4, 96], BF16, tag="diag")
                    nc.gpsimd.affine_select(
                        out=diag4,
                        in_=rDg_bf[:, w00 : w00 + 4]
                        .unsqueeze(2)
                        .broadcast_to((96, 4, 96)),
                        compare_op=AluOpType.is_equal,
                        fill=0.0,
                        base=0,
                        pattern=[[0, 4], [-1, 96]],
                        channel_multiplier=1,
                    )
                    pat = pp.tile([96, 4, 96], F32, tag="pCE")
                    for wi in range(4):
                        nc.tensor.matmul(
                            pat[:, wi, :],
                            expH[:, w00 + wi, :],
                            diag4[:, wi, :],
                            start=True,
                            stop=True,
                        )
                    attT = wk.tile([96, 4, 96], F16, tag="attT")
                    if g2 % 2 == 0:
                        nc.scalar.copy(attT, pat)
                    else:
                        nc.vector.tensor_copy(attT, pat)
                    for gg in range(2):
                        g = 2 * g2 + gg
                        w0 = 4 * g2 + 2 * gg
                        pvt = pp.tile([96, 2, KCH, 128], F16, tag="pT")
                        for wi in range(2):
                            for k in range(KCH):
                                nc.tensor.transpose(
                                    pvt[:, wi, k, :], v_sb[:, k, :, w0 + wi], id128
                                )
                        vt1 = wkv.tile([96, 2, KCH, 128], F16, tag="vt")
                        if g % 2 == 0:
                            nc.vector.tensor_copy(vt1, pvt)
                        else:
                            nc.scalar.copy(vt1, pvt)
                        pagg = pp.tile([128, KCH, 2, 128], F32, tag="pAGG")
                        for m in range(KCH):
                            for wi in range(2):
                                nc.tensor.matmul(
                                    pagg[:, m, wi, 0:96],
                                    vt1[:, wi, m, :],
                                    attT[:, 2 * gg + wi, :],
                                    start=True,
                                    stop=True,
                                )
                        srcA = pagg[:, :, :, 0:96].rearrange("p m wi h -> p m h wi")
                        dstA = o_col[:, :, :, w0 : w0 + 2]
                        if g % 2 == 0:
                            nc.scalar.copy(dstA, srcA)
                        else:
                            nc.vector.tensor_copy(dstA, srcA)

                if dbg and b == 0:
                    nc.sync.dma_start(
                        out=dbg_ocol[:],
                        in_=o_col.rearrange("p m h w -> p m (h w)").rearrange(
                            "p m (h w) -> p m h w", w=W
                        ),
                    )
                # ---------- Phase R: row pass (quads = 2 pairs of rows) ----------
                for q in range(H // 4):
                    h0q = 4 * q
                    orow = st.tile([128, KCH, 4, 96], F16, tag="orow")
                    xrow = st.tile([128, KCH, 384], F16, tag="xs")
                    nc.sync.dma_start(
                        out=xrow,
                        in_=bass.AP(
                            tensor=x16ap.tensor,
                            offset=x16ap.offset + b * C * HW + h0q * W,
                            ap=[[HW, 128], [128 * HW, KCH], [1, 384]],
                        ),
                    )
                    diag4r = wk.tile([96, 4, 96], BF16, tag="diag")
                    nc.gpsimd.affine_select(
                        out=diag4r,
                        in_=rDgT_bf[:, h0q : h0q + 4]
                        .unsqueeze(2)
                        .broadcast_to((96, 4, 96)),
                        compare_op=AluOpType.is_equal,
                        fill=0.0,
                        base=0,
                        pattern=[[0, 4], [-1, 96]],
                        channel_multiplier=1,
                    )
                    pat2 = pp.tile([96, 4, 96], F32, tag="pCE")
                    for hi in range(4):
                        nc.tensor.matmul(
                            pat2[:, hi, :],
                            expW[:, h0q + hi, :],
                            diag4r[:, hi, :],
                            start=True,
                            stop=True,
                        )
                    attT2 = wk.tile([96, 4, 96], F16, tag="attT")
                    nc.scalar.copy(attT2, pat2)
                    for p in range(2):
                        h0 = h0q + 2 * p
                        pvt = pp.tile([96, 2, KCH, 128], F16, tag="pT")
                        for hi in range(2):
                            for k in range(KCH):
                                nc.tensor.transpose(
                                    pvt[:, hi, k, :], v_sb[:, k, h0 + hi, :], id128
                                )
                        vt2 = wkv.tile([96, 2, KCH, 128], F16, tag="vt")
                        nc.scalar.copy(vt2, pvt)
                        pagg2 = pp.tile([128, KCH, 2, 128], F32, tag="pAGG")
                        for m in range(KCH):
                            for hi in range(2):
                                nc.tensor.matmul(
                                    pagg2[:, m, hi, 0:96],
                                    vt2[:, hi, m, :],
                                    attT2[:, 2 * p + hi, :],
                                    start=True,
                                    stop=True,
                                )
                        odst = orow[:, :, 2 * p : 2 * p + 2, :]
                        xsl = xrow.rearrange("p m (hi w) -> p m hi w", hi=4)[
                            :, :, 2 * p : 2 * p + 2, :
                        ]
                        # orow = (pagg2 + o_col_rows) + xrow: two DVE passes
                        # (the o_col accumulation moved off PE - identity
                        # matmuls cost ~81ns each on HW at N=96)
                        nc.vector.scalar_tensor_tensor(
                            out=odst,
                            in0=pagg2[:, :, :, 0:96],
                            scalar=1.0,
                            in1=o_col[:, :, h0 : h0 + 2, :],
                            op0=AluOpType.mult,
                            op1=AluOpType.add,
                        )
                        nc.vector.tensor_tensor(
                            out=odst, in0=odst, in1=xsl, op=AluOpType.add
                        )
                    nc.sync.dma_start(
                        out=bass.AP(
                            tensor=outap.tensor,
                            offset=outap.offset + b * C * HW + h0q * W,
                            ap=[[HW, 128], [128 * HW, KCH], [1, 384]],
                        ),
                        in_=orow.rearrange("p m hi w -> p m (hi w)"),
                    )
    nc.finalize()
    return nc


_NC_CACHE = {}


def _get_nc():
    if "nc" not in _NC_CACHE:
        _NC_CACHE["nc"] = build_nc()
    return _NC_CACHE["nc"]


def make_in_maps(x, Wq, bq, Wk, bk, Wv, bv, gamma):
    x = np.asarray(x, dtype=np.float32)
    gamma = np.asarray(gamma, dtype=np.float32)
    wqkT = np.ascontiguousarray(
        np.concatenate([np.asarray(Wq), np.asarray(Wk)], axis=0).T
    ).astype(np.float16)
    wvT = np.ascontiguousarray(np.asarray(Wv).T).astype(np.float16)
    bqk = np.concatenate([np.asarray(bq), np.asarray(bk)]).astype(np.float32)
    bv = np.asarray(bv, dtype=np.float32)
    x16 = x.astype(np.float16)
    in_maps = []
    for c in range(NCORES):
        sl = slice(c * BLOC, (c + 1) * BLOC)
        in_maps.append(
            {
                "x16": x16[sl],
                "wqkT": wqkT,
                "wvT": wvT,
                "bqk": bqk,
                "bv": bv,
                "gamma": gamma,
            }
        )
    return in_maps


def kernel(x, Wq, bq, Wk, bk, Wv, bv, gamma):
    from concourse.bass_utils import run_bass_kernel_spmd

    nc = _get_nc()
    in_maps = make_in_maps(x, Wq, bq, Wk, bk, Wv, bv, gamma)
    res = run_bass_kernel_spmd(nc, in_maps, core_ids=list(range(NCORES)))
    return np.concatenate([r["out"] for r in res.results], axis=0).astype(np.float32)
